# revision 26
# baseline (speedup 1.0000x reference)
"""Trainium2 Bass kernel for nn_CDCL_47906065219864 (semi-supervised
segmentation loss: 3-layer conv extractor + classifier/projector heads +
CE/entropy/consistency/contrastive terms -> scalar loss).

Sharding (8 cores, uniform SPMD program, per-core data):
  core c: image b = c % 4, half = c // 4 (0 = top, 1 = bottom of feature map).
  Each core runs the extractor on its unlabeled half-image first, projects +
  L2-normalizes its 800 anchors, AllGathers the bf16 student feature bank,
  overlaps the gather with (a) contrastive exp over the EMA half of the bank
  (available locally) and (b) the supervised conv branch + CE head, then
  finishes the student-half contrastive scores and the per-anchor assembly.
  NUM_CLASSES=2 collapses the classifier to the logit difference d:
     nll = softplus(d) - label*d,  H = softplus(d) - d*sigmoid(d),
     pseudo_label = (d > 0).
  Contrastive per anchor a (labels binary): with s = (a@bank)/TEMP,
     sum_b pos*logp = (1-la)*[2(T-T1) - (M-N1)L] + la*[2 T1 - N1 L],
  where T = a@Bsum, T1 = a@B1sum (bank sums), L = logsumexp_b s.
  Score layout: [128 bank rows x 800 anchors] per chunk, exp on the full
  128 partitions, Z reduced over the bank via ones-matmul PSUM accumulation.
  Convs run in bf16 (loss impact measured ~1e-5; gate is 2e-2).
Host only shards/reshapes inputs (im2col for conv1, masks, EMA argmax)
and sums the 8 cores' partial-loss vectors into the final scalar.
"""

import os
import numpy as np
import ml_dtypes

F = np.float32
BF = ml_dtypes.bfloat16

# ---------------- geometry tables ----------------
# local row counts; conv1 rows padded to 88 so the 4-block im2col layout
# splits evenly (4 x 22 rows). Real rows: sup 87, ul 85.
NL1P = 88
SUP_NL2, SUP_NF = 43, 21
UL_NL2, UL_NF = 41, 20
SUP_L1START = (0, 76)   # per half
UL_L1START = (0, 80)
SUP_FSTART = (0, 19)
UL_FSTART = (0, 20)

SHIFT = 16.0
INV_TEMP = 2.0
M_BANK = 12800.0

_CACHE = {}


# ---------------- host-side prep ----------------

def _resize_mat(oh, ih):
    Mx = np.zeros((oh, ih), F)
    s = np.linspace(0.0, ih - 1.0, oh)
    y0 = np.floor(s).astype(int)
    y1 = np.minimum(y0 + 1, ih - 1)
    w = (s - y0).astype(F)
    for i in range(oh):
        Mx[i, y0[i]] += 1 - w[i]
        Mx[i, y1[i]] += w[i]
    return Mx


def _im2col(img, l1start):
    """img [3,320,320] -> [128, 7040] bf16 for conv1 (stride2, SAME).

    K padded 27->32, rows padded to 88, laid out as 2 partition blocks of
    44 rows each: partition 64*j + k holds K-index k of rows [44j, 44j+44)
    (PE matmul base partitions must be 0/32/64).
    """
    xp = np.zeros((3, 2 * (l1start + NL1P) + 2, 322), F)
    h = min(320, xp.shape[1])
    xp[:, :h, :320] = img[:, :h]
    out = np.zeros((32, NL1P, 160), F)
    for c in range(3):
        for dy in range(3):
            for dx in range(3):
                sub = xp[c, 2 * l1start + dy: 2 * l1start + dy + 2 * NL1P:2,
                         dx: dx + 320:2]
                out[c * 9 + dy * 3 + dx] = sub
    blk = out.reshape(32, 2, 44, 160).transpose(1, 0, 2, 3).reshape(2, 32, 7040)
    ob = np.zeros((128, 7040), F)
    ob[0:32] = blk[0]
    ob[64:96] = blk[1]
    return np.ascontiguousarray(ob.astype(BF))


def _prep(inputs):
    x_l = np.ascontiguousarray(np.asarray(inputs['x_l'], F))
    y_l = np.asarray(inputs['y_l'])
    x_ul = np.ascontiguousarray(np.asarray(inputs['x_ul'], F))
    proj_ema = np.ascontiguousarray(np.asarray(inputs['proj_ul_ema'], F))
    z_ema = np.asarray(inputs['z_ul_ema'], F)
    W1 = np.asarray(inputs['W1'], F); W2 = np.asarray(inputs['W2'], F)
    W3 = np.asarray(inputs['W3'], F)
    Wc = np.asarray(inputs['Wc'], F)[:, :, 0, 0]
    bc = np.asarray(inputs['bc'], F)
    Wp = np.asarray(inputs['Wp'], F)[:, :, 0, 0]

    Ry = _resize_mat(320, 40)
    Rx = _resize_mat(320, 40)
    wc_d = Wc[1] - Wc[0]
    bc_d = F(bc[1] - bc[0])

    # EMA prep (input-only)
    pl_ema = np.argmax(z_ema, axis=1).astype(F)                # [4,40,40]
    e = z_ema - z_ema.max(axis=1, keepdims=True)
    p_ema = np.exp(e) / np.exp(e).sum(axis=1, keepdims=True)
    mask_ema = (p_ema.max(axis=1) > 0.6).astype(F)             # [4,40,40]
    labE = pl_ema.reshape(4, 1600)
    ebank = np.concatenate([proj_ema[b].reshape(128, 1600) for b in range(4)], axis=1)
    labEf = np.concatenate([labE[b] for b in range(4)])
    epack = np.stack([ebank.sum(1), (ebank * labEf[None]).sum(1)], axis=1).astype(F)
    en1 = np.array([[labEf.sum()]], F)

    w1m = np.zeros((128, 64), F)
    w1m[0:27] = W1.transpose(1, 2, 3, 0).reshape(27, 64)
    w1m[64:91] = w1m[0:27]
    w1b = np.ascontiguousarray(w1m.astype(BF))  # [128,64], blocks at 0 and 64
    w2b = np.ascontiguousarray(
        W2.transpose(1, 2, 3, 0).reshape(64, 9 * 128).astype(BF))
    w3b = np.ascontiguousarray(
        W3.transpose(1, 2, 3, 0).reshape(128, 9 * 256).astype(BF))
    wpt = np.ascontiguousarray(
        Wp.T.reshape(2, 128, 128).transpose(1, 0, 2).reshape(128, 256))
    wcd = np.ascontiguousarray(wc_d.reshape(2, 128).T)         # [128,2]
    bcd = np.array([[bc_d]], F)
    rxt = np.ascontiguousarray(Rx.T)                           # [40,320]

    shared = dict(w1b=w1b, w2b=w2b, w3b=w3b, wpt=wpt, wcd=wcd, bcd=bcd,
                  rxt=rxt, ebank=np.ascontiguousarray(ebank), epack=epack,
                  en1=en1)

    in_maps = []
    for c in range(8):
        b, half = c % 4, c // 4
        xs = _im2col(x_l[b], SUP_L1START[half])
        xu = _im2col(x_ul[b], UL_L1START[half])
        f0 = SUP_FSTART[half]
        ryt = np.ascontiguousarray(
            Ry[160 * half:160 * half + 160, f0:f0 + SUP_NF].T)  # [21,160]
        y_h = y_l[b, 160 * half:160 * half + 160]                # [160,320]
        vmask = (y_h != 255).astype(F)
        yf = np.clip(y_h, 0, 1).astype(F)
        supA = vmask
        supB = yf * vmask
        u0 = UL_FSTART[half]
        plm = (pl_ema[b, u0:u0 + 20] * mask_ema[b, u0:u0 + 20]).reshape(100, 8)
        mkm = mask_ema[b, u0:u0 + 20].reshape(100, 8)
        m = dict(shared)
        m.update(xs=xs, xu=xu, ryt=ryt, supA=np.ascontiguousarray(supA),
                 supB=np.ascontiguousarray(supB),
                 plm=np.ascontiguousarray(plm), mkm=np.ascontiguousarray(mkm))
        in_maps.append(m)

    meta = dict(sup_cnt=float((y_l != 255).sum()),
                cons_cnt=float(mask_ema.sum()),
                epoch=int(np.asarray(inputs['epoch'])))
    return in_maps, meta


def _combine(rows, meta):
    s = np.asarray(rows, np.float64).sum(axis=0)
    # slots 0/1/4/6 accumulate q = ln(sigmoid(-z)) = -softplus(z) products
    sup_nll = -(s[0] + s[1]) - (s[2] + s[3])
    ent = -s[4] - s[5]
    cons = -s[6] - s[7]
    contr_num, inc_sum = s[12], s[13]
    loss_sup = sup_nll / max(meta['sup_cnt'], 1.0)
    epoch = meta['epoch']
    if epoch < 5:
        return np.float32(loss_sup)
    loss_ent = ent / 6400.0
    loss_cons = cons / max(meta['cons_cnt'], 1.0)
    loss_contr = contr_num / max(inc_sum, 1.0)
    ramp = min(max(epoch / 40.0, 0.0), 1.0)
    cons_w = 1.0 * float(np.exp(-5.0 * (1.0 - ramp) ** 2))
    return np.float32(loss_sup + 0.1 * loss_contr + cons_w * loss_cons
                      + 0.01 * loss_ent)


# ---------------- bass program ----------------

def _build():
    import concourse.bacc as bacc
    import concourse.bass as bass
    import concourse.mybir as mybir
    from concourse import tile

    dt = mybir.dt
    F32 = dt.float32
    BF16 = dt.bfloat16
    AF = mybir.ActivationFunctionType
    OP = mybir.AluOpType

    nc = bacc.Bacc("TRN2", target_bir_lowering=False, debug=False,
                   num_devices=8)

    # ---- dram I/O ----
    din = {}
    for name, shape, dtp in [
            ('xs', [128, 7040], BF16), ('xu', [128, 7040], BF16),
            ('ryt', [21, 160], F32), ('rxt', [40, 320], F32),
            ('supA', [160, 320], F32), ('supB', [160, 320], F32),
            ('plm', [100, 8], F32), ('mkm', [100, 8], F32),
            ('w1b', [128, 64], BF16), ('w2b', [64, 9 * 128], BF16),
            ('w3b', [128, 9 * 256], BF16),
            ('wpt', [128, 256], F32), ('wcd', [128, 2], F32),
            ('bcd', [1, 1], F32),
            ('ebank', [128, 6400], F32), ('epack', [128, 2], F32),
            ('en1', [1, 1], F32)]:
        din[name] = nc.dram_tensor(name, shape, dtp, kind="ExternalInput")
    dout = nc.dram_tensor('part', [1, 16], F32, kind="ExternalOutput")

    eye_np = np.eye(128, dtype=np.float32)
    ones_np = np.ones((128, 128), dtype=np.float32)
    eye_d = nc.inline_tensor(eye_np, name='eye_c')
    ones_d = nc.inline_tensor(ones_np, name='ones_c')

    from contextlib import ExitStack
    with tile.TileContext(nc) as tc, ExitStack() as _es:
        cpool = _es.enter_context(tc.tile_pool(name="consts", bufs=1))
        big = _es.enter_context(tc.tile_pool(name="big", bufs=1))
        work = _es.enter_context(tc.tile_pool(name="work", bufs=1))
        wk2 = _es.enter_context(tc.tile_pool(name="wk2", bufs=2))
        dram = _es.enter_context(tc.tile_pool(name="dram", bufs=1, space="DRAM"))
        smallps = _es.enter_context(tc.tile_pool(name="smallps", bufs=2, space="PSUM"))

        # ---- consts / weights to SBUF ----
        def load(name):
            src = din[name]
            t = cpool.tile(list(src.shape), src.dtype, name=f"{name}_sb")
            nc.sync.dma_start(t[:], src[:])
            return t

        eye_sb = cpool.tile([128, 128], F32, name="eye_sb")
        nc.sync.dma_start(eye_sb[:], eye_d[:])
        ones_sb = cpool.tile([128, 128], F32, name="ones_sb")
        nc.sync.dma_start(ones_sb[:], ones_d[:])

        # bank first: EMA half streams in from DRAM immediately (no deps)
        bank = big.tile([128, 12800], BF16, tag="bank", name="bank")
        sbank = bank[:, 0:6400]
        ebsb = bank[:, 6400:12800]
        for i in range(4):
            nc.gpsimd.dma_start(ebsb[:, i * 1600:(i + 1) * 1600],
                                din['ebank'][:, i * 1600:(i + 1) * 1600])

        w1sb = load('w1b')
        w2sb = load('w2b')     # [64, tap*128] bf16
        w3sb = load('w3b')     # [128, tap*256] bf16
        wpsb = load('wpt')     # [128, half*128]
        wcsb = load('wcd')
        bcdsb = load('bcd')
        rxsb = load('rxt')
        rysb = load('ryt')
        epsb = load('epack')
        en1sb = load('en1')
        plsb = load('plm')
        mksb = load('mkm')

        acc = work.tile([128, 12], F32, name="acc")
        nc.vector.memset(acc[:], 0.0)
        shiftsb = cpool.tile([128, 1], F32, name="shiftsb")
        nc.vector.memset(shiftsb[:], -SHIFT)
        ones_bf = cpool.tile([128, 1], BF16, name="ones_bf")
        nc.vector.memset(ones_bf[:], 1.0)
        ones8 = cpool.tile([8, 1], F32, name="ones8")
        nc.vector.memset(ones8[:], 1.0)

        # ================= conv stack (bf16) =================
        def conv_stack(xdram, n_l2, n_fea, tagp):
            with tc.tile_pool(name=f"convps{tagp}", bufs=6, space="PSUM") as cps:
                l1t = big.tile([64, NL1P, 161], BF16, tag="l1", name=f"l1{tagp}")
                nc.vector.memset(l1t[:, :, 160:161], 0.0)
                xt = big.tile([128, 7040], BF16, tag="x", name=f"x{tagp}",
                              bufs=2)
                nc.sync.dma_start(xt[:], xdram[:])
                # conv1: 2 partition blocks of 44 rows (bases 0 and 64)
                ri = 0
                for j in range(2):
                    c0 = 0
                    while c0 < 7040:
                        n = min(480, 7040 - c0)
                        nr = n // 160
                        g0 = 44 * j + c0 // 160
                        ps = cps.tile([128, 480], F32, tag="cps", name="ps1")
                        nc.tensor.matmul(ps[:64, :n],
                                         w1sb[64 * j:64 * j + 32, :],
                                         xt[64 * j:64 * j + 32, c0:c0 + n],
                                         start=True, stop=True)
                        eng = nc.vector if ri % 2 == 0 else nc.gpsimd
                        eng.tensor_scalar_max(
                            l1t[:, g0: g0 + nr, 0:160],
                            ps[:64, :n].rearrange("p (r x) -> p r x", x=160),
                            0.0)
                        ri += 1
                        c0 += n
                # conv2: 6-row chunks, 9 taps accumulate
                l2t = big.tile([128, SUP_NL2, 81], BF16, tag="l2",
                               name=f"l2{tagp}")
                nc.vector.memset(l2t[:, :, 80:81], 0.0)
                r0 = 0
                while r0 < n_l2:
                    nr = min(6, n_l2 - r0)
                    ps = cps.tile([128, 480], F32, tag="cps", name="ps2")
                    pv = ps[:, :nr * 80].rearrange("p (r x) -> p r x", x=80)
                    for tap in range(9):
                        dy, dxx = tap // 3, tap % 3
                        rhs = l1t[:, 2 * r0 + dy: 2 * r0 + dy + 2 * nr - 1:2,
                                  dxx: dxx + 159:2]
                        nc.tensor.matmul(pv, w2sb[:, tap * 128:(tap + 1) * 128],
                                         rhs, start=(tap == 0), stop=(tap == 8))
                    eng = nc.vector if (r0 // 6) % 2 == 0 else nc.gpsimd
                    eng.tensor_scalar_max(l2t[:, r0:r0 + nr, 0:80], pv, 0.0)
                    r0 += nr
                # conv3: 12-row chunks, 2 M-halves, 9 taps
                feas = []
                for h in range(2):
                    ft = big.tile([128, 840], F32, tag=f"fea{tagp}{h}",
                                  name=f"fea{tagp}{h}")
                    r0 = 0
                    while r0 < n_fea:
                        nr = min(12, n_fea - r0)
                        ps = cps.tile([128, 480], F32, tag="cps", name="ps3")
                        pv = ps[:, :nr * 40].rearrange("p (r x) -> p r x", x=40)
                        for tap in range(9):
                            dy, dxx = tap // 3, tap % 3
                            rhs = l2t[:, 2 * r0 + dy: 2 * r0 + dy + 2 * nr - 1:2,
                                      dxx: dxx + 79:2]
                            nc.tensor.matmul(
                                pv, w3sb[:, tap * 256 + h * 128:
                                         tap * 256 + h * 128 + 128],
                                rhs, start=(tap == 0), stop=(tap == 8))
                        eng = nc.vector if (h + r0) % 2 == 0 else nc.gpsimd
                        eng.tensor_scalar_max(
                            ft[:, r0 * 40:(r0 + nr) * 40],
                            ps[:, :nr * 40], 0.0)
                        r0 += nr
                    feas.append(ft)
            return feas

        with nc.named_scope("conv_ul"):
            fu_lo, fu_hi = conv_stack(din['xu'], UL_NL2, UL_NF, "u")

        # ================= ul projection + payload + AllGather =================
        with nc.named_scope("proj"):
            proj_raw = work.tile([128, 800], F32, name="proj_raw")
            fuv = [t[:, 0:800].rearrange("p (y x) -> p y x", x=40)
                   for t in (fu_lo, fu_hi)]
            for ci in range(2):
                psp = smallps.tile([128, 400], F32, tag="sp", name="psp")
                for pp in range(4):
                    p = ci * 4 + pp
                    jl, kk = p // 4, p % 4
                    for h in range(2):
                        nc.tensor.matmul(
                            psp[:, pp * 100:(pp + 1) * 100],
                            wpsb[:, h * 128:(h + 1) * 128],
                            fuv[h][:, 10 * jl:10 * jl + 10,
                                   10 * kk:10 * kk + 10],
                            start=(h == 0), stop=(h == 1))
                nc.scalar.copy(proj_raw[:, ci * 400:(ci + 1) * 400], psp[:])
            # d_ul row + labels
            du_row = work.tile([1, 800], F32, name="du_row")
            for ci in range(2):
                psd = smallps.tile([1, 400], F32, tag="sp", name="psd")
                for h in range(2):
                    nc.tensor.matmul(psd[:], wcsb[:, h:h + 1],
                                     (fu_lo, fu_hi)[h][:, ci * 400:(ci + 1) * 400],
                                     start=(h == 0), stop=(h == 1))
                nc.vector.tensor_scalar(du_row[:, ci * 400:(ci + 1) * 400],
                                        psd[:], bcdsb[0:1, 0:1], None,
                                        op0=OP.add)
            la_row = work.tile([1, 800], F32, name="la_row")
            nc.vector.tensor_scalar(la_row[:], du_row[:], 0.0, None, op0=OP.is_gt)
            la_pat = work.tile([1, 800], F32, name="la_pat")
            lrv = la_row[:].rearrange("p (y x) -> p y x", x=40)
            for p in range(8):
                jl, kk = p // 4, p % 4
                nc.sync.dma_start(
                    la_pat[:, p * 100:(p + 1) * 100].rearrange(
                        "p (y x) -> p y x", x=10),
                    lrv[:, 10 * jl:10 * jl + 10, 10 * kk:10 * kk + 10])
            # normalize proj
            sq = work.tile([128, 800], F32, name="sq")
            nc.vector.tensor_mul(sq[:], proj_raw[:], proj_raw[:])
            nrm = work.tile([1, 800], F32, name="nrm")
            for ci in range(2):
                pss = smallps.tile([1, 400], F32, tag="sp", name="pss")
                nc.tensor.matmul(pss[:], ones_sb[:, 0:1],
                                 sq[:, ci * 400:(ci + 1) * 400],
                                 start=True, stop=True)
                nc.scalar.activation(nrm[:, ci * 400:(ci + 1) * 400], pss[:],
                                     AF.Sqrt)
            nc.vector.tensor_scalar_max(nrm[:], nrm[:], 1e-12)
            inv = work.tile([1, 800], F32, name="inv")
            nc.vector.reciprocal(inv[:], nrm[:])
            inv_bc = work.tile([128, 800], F32, tag="bc800", name="inv_bc")
            for ci in range(2):
                psb1 = smallps.tile([128, 400], F32, tag="sp", name="psb1")
                nc.tensor.matmul(psb1[:], ones_sb[0:1, :],
                                 inv[:, ci * 400:(ci + 1) * 400],
                                 start=True, stop=True)
                nc.scalar.copy(inv_bc[:, ci * 400:(ci + 1) * 400], psb1[:])
            proj_n = work.tile([128, 800], F32, name="proj_n")
            nc.vector.tensor_mul(proj_n[:], proj_raw[:], inv_bc[:])
            proj_nb = work.tile([128, 800], BF16, name="proj_nb")
            nc.vector.tensor_copy(proj_nb[:], proj_n[:])
            la_bc = work.tile([128, 800], F32, tag="bc800", name="la_bc")
            for ci in range(2):
                psb2 = smallps.tile([128, 400], F32, tag="sp", name="psb2")
                nc.tensor.matmul(psb2[:], ones_sb[0:1, :],
                                 la_pat[:, ci * 400:(ci + 1) * 400],
                                 start=True, stop=True)
                nc.scalar.copy(la_bc[:, ci * 400:(ci + 1) * 400], psb2[:])
            vsum = work.tile([128, 1], F32, name="vsum")
            nc.vector.tensor_reduce(vsum[:], proj_n[:], mybir.AxisListType.X, OP.add)
            v1 = work.tile([128, 1], F32, name="v1")
            nc.vector.scalar_tensor_tensor(sq[:], proj_n[:], 1.0, la_bc[:],
                                           op0=OP.mult, op1=OP.mult, accum_out=v1[:])
            n1loc = work.tile([1, 1], F32, name="n1loc")
            nc.vector.tensor_reduce(n1loc[:], la_row[:], mybir.AxisListType.X, OP.add)

            pay = dram.tile([128, 804], BF16, name="pay")
            gath = dram.tile([1024, 804], BF16, name="gath", addr_space="Shared")
            zpad = cpool.tile([128, 2], BF16, name="zpad")
            nc.vector.memset(zpad[:], 0.0)
            nc.sync.dma_start(pay[:, 802:804], zpad[:])
            nc.sync.dma_start(pay[:, 0:800], proj_nb[:])
            nc.gpsimd.dma_start(pay[:, 800:801], vsum[:])
            nc.gpsimd.dma_start(pay[:, 801:802], v1[:])
            nc.gpsimd.dma_start(pay[0:1, 802:803], n1loc[:])
            nc.gpsimd.collective_compute(
                "AllGather", OP.bypass,
                replica_groups=[list(range(8))],
                ins=[pay[:].opt()], outs=[gath[:].opt()])

        # ================= EMA-half contrastive scores (overlap AllGather) ====
        with nc.named_scope("contr_ema"), \
                tc.tile_pool(name="emaps", bufs=1, space="PSUM") as eps:
            Ze = work.tile([1, 800], F32, name="Ze")
            zpe = eps.tile([1, 800], F32, tag="z", name="zpe")
            for c in range(50):
                sps = eps.tile([128, 800], F32, tag="cs", name="sps", bufs=2)
                for h2, (a0, a1) in enumerate(((0, 512), (512, 800))):
                    nc.tensor.matmul(sps[:, a0:a1],
                                     ebsb[:, c * 128:(c + 1) * 128],
                                     proj_nb[:, a0:a1],
                                     start=True, stop=True)
                et = wk2.tile([128, 800], BF16, tag="et", name="et", bufs=3)
                nc.scalar.activation(et[:], sps[:], AF.Exp,
                                     scale=INV_TEMP, bias=shiftsb[:, 0:1])
                for h2, (a0, a1) in enumerate(((0, 512), (512, 800))):
                    nc.tensor.matmul(zpe[:, a0:a1], ones_bf[:], et[:, a0:a1],
                                     start=(c == 0), stop=(c == 49))
            nc.vector.tensor_copy(Ze[:], zpe[:])

        # ================= sup branch (overlaps AllGather) =================
        with nc.named_scope("conv_sup"):
            fs_lo, fs_hi = conv_stack(din['xs'], SUP_NL2, SUP_NF, "s")

        with nc.named_scope("sup_head"):
            dsup = work.tile([1, 840], F32, name="dsup")
            for ci in range(2):
                psd2 = smallps.tile([1, 420], F32, tag="sp", name="psd2")
                for h in range(2):
                    nc.tensor.matmul(psd2[:], wcsb[:, h:h + 1],
                                     (fs_lo, fs_hi)[h][:, ci * 420:(ci + 1) * 420],
                                     start=(h == 0), stop=(h == 1))
                nc.vector.tensor_scalar(dsup[:, ci * 420:(ci + 1) * 420],
                                        psd2[:], bcdsb[0:1, 0:1], None,
                                        op0=OP.add)
            d_yx = work.tile([21, 40], F32, name="d_yx")
            nc.sync.dma_start(d_yx[:], dsup[:].rearrange("p (y x) -> p y x", x=40))
            pstr = smallps.tile([40, 21], F32, tag="sp", name="pstr")
            nc.tensor.transpose(pstr[:], d_yx[:], eye_sb[0:21, 0:21])
            dT = work.tile([40, 21], F32, name="dT")
            nc.scalar.copy(dT[:], pstr[:])
            pst1 = smallps.tile([21, 320], F32, tag="sp", name="pst1")
            nc.tensor.matmul(pst1[:], dT[:], rxsb[:], start=True, stop=True)
            tmp1 = work.tile([21, 320], F32, name="tmp1")
            nc.scalar.copy(tmp1[:], pst1[:])
            # z chunks + CE partials
            for ci, (p0, npp) in enumerate([(0, 128), (128, 32)]):
                psz = smallps.tile([128, 320], F32, tag="sp", name="psz")
                nc.tensor.matmul(psz[:npp, :], rysb[:, p0:p0 + npp],
                                 tmp1[:], start=True, stop=True)
                asb = wk2.tile([128, 320], F32, tag="ab", name="asb")
                bsb = wk2.tile([128, 320], F32, tag="ab", name="bsb")
                nc.sync.dma_start(asb[:npp, :], din['supA'][p0:p0 + npp, :])
                nc.sync.dma_start(bsb[:npp, :], din['supB'][p0:p0 + npp, :])
                sp = wk2.tile([128, 320], F32, tag="sp2", name="spz")
                nc.scalar.activation(sp[:npp, :], psz[:npp, :], AF.Sigmoid,
                                     scale=-1.0)
                nc.scalar.activation(sp[:npp, :], sp[:npp, :], AF.Ln)
                jk = wk2.tile([128, 320], F32, tag="jk", name="jk")
                nc.vector.scalar_tensor_tensor(
                    jk[:npp, :], sp[:npp, :], 1.0, asb[:npp, :],
                    op0=OP.mult, op1=OP.mult,
                    accum_out=acc[0:npp, 0 + ci:1 + ci])
                nc.vector.scalar_tensor_tensor(
                    jk[:npp, :], psz[:npp, :], 1.0, bsb[:npp, :],
                    op0=OP.mult, op1=OP.mult,
                    accum_out=acc[0:npp, 2 + ci:3 + ci])

        # ================= ul head =================
        with nc.named_scope("ul_head"):
            dut = work.tile([100, 8], F32, name="dut")
            nc.sync.dma_start(dut[:], du_row[:].rearrange("p (a b) -> p a b", b=8))
            spu = work.tile([100, 8], F32, name="spu")
            nc.scalar.activation(spu[:], dut[:], AF.Sigmoid, scale=-1.0)
            nc.scalar.activation(spu[:], spu[:], AF.Ln,
                                 accum_out=acc[0:100, 4:5])
            sgu = work.tile([100, 8], F32, name="sgu")
            nc.scalar.activation(sgu[:], dut[:], AF.Sigmoid)
            jk2 = work.tile([100, 8], F32, name="jk2")
            nc.vector.scalar_tensor_tensor(jk2[:], dut[:], 1.0, sgu[:],
                                           op0=OP.mult, op1=OP.mult,
                                           accum_out=acc[0:100, 5:6])
            nc.vector.scalar_tensor_tensor(jk2[:], spu[:], 1.0, mksb[:],
                                           op0=OP.mult, op1=OP.mult,
                                           accum_out=acc[0:100, 6:7])
            nc.vector.scalar_tensor_tensor(jk2[:], dut[:], 1.0, plsb[:],
                                           op0=OP.mult, op1=OP.mult,
                                           accum_out=acc[0:100, 7:8])
            # per-patch label layout [8, 100] + counts [8, 1]
            la_p8 = work.tile([8, 100], F32, name="la_p8")
            nc.sync.dma_start(la_p8[:],
                              la_pat[:].rearrange("p (b a) -> p b a", a=100))
            la_cnt = work.tile([8, 1], F32, name="la_cnt")
            nc.vector.tensor_reduce(la_cnt[:], la_p8[:],
                                    mybir.AxisListType.X, OP.add)

        # ================= post-AG bank assembly =================
        with nc.named_scope("bank"):
            gv = gath[:].rearrange("(r p) c -> r p c", p=128)
            for r in range(8):
                nc.gpsimd.dma_start(sbank[:, r * 800:(r + 1) * 800],
                                    gv[r, :, 0:800])
            BB = work.tile([128, 2], F32, name="BB")
            nc.vector.tensor_copy(BB[:], epsb[:])
            for r in range(8):
                vv = wk2.tile([128, 2], F32, tag="vv", name="vv")
                nc.gpsimd.dma_start(vv[:], gv[r, :, 800:802])
                nc.vector.tensor_add(BB[:], BB[:], vv[:])
            n8 = work.tile([1, 8], F32, name="n8")
            nc.gpsimd.dma_start(n8[:], gv[:, 0:1, 802:803].rearrange(
                "r p c -> p (r c)"))
            n1t = work.tile([1, 1], F32, name="n1t")
            nc.vector.tensor_reduce(n1t[:], n8[:], mybir.AxisListType.X, OP.add)
            nc.vector.tensor_add(n1t[:], n1t[:], en1sb[:])
            MN1s = work.tile([1, 1], F32, name="MN1s")
            nc.vector.tensor_scalar(MN1s[:], n1t[:], -1.0, M_BANK,
                                    op0=OP.mult, op1=OP.add)
            DNs = work.tile([1, 1], F32, name="DNs")
            nc.vector.tensor_scalar(DNs[:], n1t[:], 2.0, -M_BANK,
                                    op0=OP.mult, op1=OP.add)
            BBb = work.tile([128, 2], BF16, name="BBb")
            nc.vector.tensor_copy(BBb[:], BB[:])

        # ================= student-half scores + assembly =================
        with nc.named_scope("contr_stu"), \
                tc.tile_pool(name="stups", bufs=1, space="PSUM") as sps_pool:
            T0_sb = work.tile([1, 800], F32, name="T0_sb")
            T1_sb = work.tile([1, 800], F32, name="T1_sb")
            ttp = sps_pool.tile([128, 800], F32, tag="cs", name="ttp", bufs=2)
            for ti, tdst in enumerate((T0_sb, T1_sb)):
                for h2, (a0, a1) in enumerate(((0, 512), (512, 800))):
                    nc.tensor.matmul(ttp[:1, a0:a1], BBb[:, ti:ti + 1],
                                     proj_nb[:, a0:a1],
                                     start=True, stop=True)
                nc.vector.tensor_copy(tdst[:], ttp[:1, :])
            zps = sps_pool.tile([1, 800], F32, tag="z", name="zps")
            for c in range(50):
                sps = sps_pool.tile([128, 800], F32, tag="cs", name="sps2",
                                    bufs=2)
                for h2, (a0, a1) in enumerate(((0, 512), (512, 800))):
                    nc.tensor.matmul(sps[:, a0:a1],
                                     sbank[:, c * 128:(c + 1) * 128],
                                     proj_nb[:, a0:a1],
                                     start=True, stop=True)
                et = wk2.tile([128, 800], BF16, tag="et", name="et2", bufs=3)
                nc.scalar.activation(et[:], sps[:], AF.Exp,
                                     scale=INV_TEMP, bias=shiftsb[:, 0:1])
                for h2, (a0, a1) in enumerate(((0, 512), (512, 800))):
                    nc.tensor.matmul(zps[:, a0:a1], ones_bf[:], et[:, a0:a1],
                                     start=(c == 0), stop=(c == 49))
            Zrow = work.tile([1, 800], F32, name="Zrow")
            nc.vector.tensor_add(Zrow[:], zps[:], Ze[:])

            # per-anchor loss assembly on [1, 800] rows
            pv = work.tile([1, 800 * 8], F32, name="pv")
            Lr, m1, U1, tm, U0, d10, pd, rr = (
                pv[:, i * 800:(i + 1) * 800] for i in range(8))
            nc.scalar.activation(Lr, Zrow[:], AF.Ln)
            nc.vector.tensor_scalar_add(Lr, Lr, SHIFT)
            nc.vector.tensor_scalar(m1, Lr, n1t[0:1, 0:1], None, op0=OP.mult)
            nc.vector.scalar_tensor_tensor(U1, T1_sb[:], INV_TEMP, m1,
                                           op0=OP.mult, op1=OP.subtract)
            nc.vector.tensor_sub(tm, T0_sb[:], T1_sb[:])
            nc.vector.tensor_scalar(m1, Lr, MN1s[0:1, 0:1], None, op0=OP.mult)
            nc.vector.scalar_tensor_tensor(U0, tm, INV_TEMP, m1,
                                           op0=OP.mult, op1=OP.subtract)
            nc.vector.tensor_sub(d10, U1, U0)
            nc.vector.scalar_tensor_tensor(pd, la_pat[:], 1.0, d10,
                                           op0=OP.mult, op1=OP.mult)
            nc.vector.tensor_add(pd, pd, U0)
            nc.vector.tensor_scalar(tm, la_pat[:], DNs[0:1, 0:1],
                                    MN1s[0:1, 0:1], op0=OP.mult, op1=OP.add)
            nc.vector.reciprocal(rr, tm)
            nc.vector.tensor_mul(pd, pd, rr)

            PA_p8 = work.tile([8, 100], F32, name="PA_p8")
            nc.sync.dma_start(PA_p8[:],
                              pd.rearrange("p (b a) -> p b a", a=100))
            pack = work.tile([8, 2], F32, name="pack")
            PAs = work.tile([8, 1], F32, name="PAs")
            nc.vector.tensor_reduce(PAs[:], PA_p8[:],
                                    mybir.AxisListType.X, OP.add)
            fc = work.tile([8, 1], F32, name="fc")
            nc.vector.tensor_scalar_mul(fc[:], la_cnt[:], 0.01)
            g1 = work.tile([8, 1], F32, name="g1")
            nc.vector.tensor_scalar(g1[:], fc[:], 0.1, None, op0=OP.is_gt)
            g2 = work.tile([8, 1], F32, name="g2")
            nc.vector.tensor_scalar(g2[:], fc[:], 0.9, None, op0=OP.is_lt)
            nc.vector.tensor_mul(g1[:], g1[:], g2[:])
            nc.vector.tensor_scalar(pack[:, 1:2], g1[:], -1.0, 1.0,
                                    op0=OP.mult, op1=OP.add)
            nc.vector.tensor_scalar_mul(PAs[:], PAs[:], -0.01)
            nc.vector.tensor_mul(pack[:, 0:1], pack[:, 1:2], PAs[:])

            psfin = smallps.tile([1, 2], F32, tag="sp", name="psfin")
            nc.tensor.matmul(psfin[:], ones8[:], pack[:], start=True, stop=True)
            outrow = work.tile([1, 16], F32, name="outrow")
            nc.vector.memset(outrow[:], 0.0)
            nc.vector.tensor_copy(outrow[:, 12:14], psfin[:])
            psacc = smallps.tile([1, 12], F32, tag="sp", name="psacc")
            nc.tensor.matmul(psacc[:], ones_sb[:, 0:1], acc[:],
                             start=True, stop=True)
            nc.scalar.copy(outrow[:, 0:12], psacc[:])
            nc.sync.dma_start(dout[:], outrow[:])

    nc.compile()
    return nc


def _get_nc():
    if 'nc' not in _CACHE:
        _CACHE['nc'] = _build()
    return _CACHE['nc']


def run_on_cores(inputs, trace=False):
    """Returns (scalar_loss, exec_time_ns_or_None)."""
    from concourse.bass_utils import run_bass_kernel_spmd
    in_maps, meta = _prep(inputs)
    nc = _get_nc()
    res = run_bass_kernel_spmd(nc, in_maps, core_ids=list(range(8)),
                               trace=trace)
    rows = [res.results[c]['part'][0] for c in range(8)]
    return _combine(rows, meta), res.exec_time_ns


def run_timed(inputs, reps=5):
    """Correctness + timing: jit once, pre-place inputs on devices, time
    repeated executions (min over reps approximates HW exec + dispatch)."""
    import time
    import jax
    import numpy as np_
    import concourse.mybir as mybir
    from jax.sharding import Mesh, PartitionSpec, NamedSharding
    from jax.experimental.shard_map import shard_map
    from concourse import bass2jax
    from concourse.bass2jax import _bass_exec_p, partition_id_tensor

    bass2jax.install_neuronx_cc_hook()
    in_maps, meta = _prep(inputs)
    nc = _get_nc()

    partition_name = nc.partition_id_tensor.name if nc.partition_id_tensor else None
    in_names, out_names, out_avals, zero_outs = [], [], [], []
    for alloc in nc.m.functions[0].allocations:
        if not isinstance(alloc, mybir.MemoryLocationSet):
            continue
        name = alloc.memorylocations[0].name
        if alloc.kind == "ExternalInput":
            if name != partition_name:
                in_names.append(name)
        elif alloc.kind == "ExternalOutput":
            out_names.append(name)
            shape = tuple(alloc.tensor_shape)
            dtype = mybir.dt.np(alloc.dtype)
            out_avals.append(jax.core.ShapedArray(shape, dtype))
            zero_outs.append(np_.zeros(shape, dtype))
    n_params = len(in_names)
    all_names = in_names + out_names + ([partition_name] if partition_name else [])

    def _body(*args):
        operands = list(args)
        if partition_name is not None:
            operands.append(partition_id_tensor())
        outs = _bass_exec_p.bind(
            *operands, out_avals=tuple(out_avals), in_names=tuple(all_names),
            out_names=tuple(out_names), lowering_input_output_aliases=(),
            sim_require_finite=True, sim_require_nnan=True, nc=nc)
        return tuple(outs)

    devices = jax.devices()[:8]
    mesh = Mesh(np_.asarray(devices), ("core",))
    spec = NamedSharding(mesh, PartitionSpec("core"))
    n_outs = len(out_names)
    sharded = jax.jit(
        shard_map(_body, mesh=mesh,
                  in_specs=(PartitionSpec("core"),) * (n_params + n_outs),
                  out_specs=(PartitionSpec("core"),) * n_outs,
                  check_rep=False),
        keep_unused=True)
    concat_in = [
        jax.device_put(np_.concatenate(
            [np_.asarray(in_maps[c][in_names[i]]) for c in range(8)], axis=0), spec)
        for i in range(n_params)]
    concat_zeros = [
        jax.device_put(np_.zeros((8 * z.shape[0], *z.shape[1:]), z.dtype), spec)
        for z in zero_outs]
    jax.block_until_ready(concat_in)

    times = []
    outs = None
    for _ in range(reps):
        t0 = time.perf_counter()
        outs = sharded(*concat_in, *concat_zeros)
        jax.block_until_ready(outs)
        times.append(time.perf_counter() - t0)
    oarr = np_.asarray(outs[out_names.index('part')]).reshape(8, *out_avals[0].shape)
    rows = [oarr[c][0] for c in range(8)]
    return _combine(rows, meta), times


def bench_chain_slope(inputs, n_small=32, n_big=256, reps=4):
    """Per-execution device time: chain N executions (each consumes the
    previous call's output buffers -> serialized on device), slope between
    n_small and n_big cancels the axon dispatch overhead."""
    import time
    import jax
    import numpy as np_
    import concourse.mybir as mybir
    from jax.sharding import Mesh, PartitionSpec, NamedSharding
    from jax.experimental.shard_map import shard_map
    from concourse import bass2jax
    from concourse.bass2jax import _bass_exec_p, partition_id_tensor

    bass2jax.install_neuronx_cc_hook()
    in_maps, meta = _prep(inputs)
    nc = _get_nc()
    pname = nc.partition_id_tensor.name if nc.partition_id_tensor else None
    in_names, out_names, out_avals, zero_outs = [], [], [], []
    for alloc in nc.m.functions[0].allocations:
        if not isinstance(alloc, mybir.MemoryLocationSet):
            continue
        name = alloc.memorylocations[0].name
        if alloc.kind == "ExternalInput":
            if name != pname:
                in_names.append(name)
        elif alloc.kind == "ExternalOutput":
            out_names.append(name)
            shape = tuple(alloc.tensor_shape)
            dtype = mybir.dt.np(alloc.dtype)
            out_avals.append(jax.core.ShapedArray(shape, dtype))
            zero_outs.append(np_.zeros(shape, dtype))
    n_params = len(in_names)
    all_names = in_names + out_names + ([pname] if pname else [])

    def _body(*args):
        operands = list(args)
        if pname:
            operands.append(partition_id_tensor())
        return tuple(_bass_exec_p.bind(
            *operands, out_avals=tuple(out_avals), in_names=tuple(all_names),
            out_names=tuple(out_names), lowering_input_output_aliases=(),
            sim_require_finite=True, sim_require_nnan=True, nc=nc))

    devices = jax.devices()[:8]
    mesh = Mesh(np_.asarray(devices), ("core",))
    spec = NamedSharding(mesh, PartitionSpec("core"))
    n_outs = len(out_names)
    sharded = jax.jit(shard_map(_body, mesh=mesh,
                                in_specs=(PartitionSpec("core"),) * (n_params + n_outs),
                                out_specs=(PartitionSpec("core"),) * n_outs,
                                check_rep=False), keep_unused=True)
    concat_in = [jax.device_put(np_.concatenate(
        [np_.asarray(in_maps[c][in_names[i]]) for c in range(8)], axis=0), spec)
        for i in range(n_params)]
    concat_zeros = [jax.device_put(
        np_.zeros((8 * z.shape[0], *z.shape[1:]), z.dtype), spec)
        for z in zero_outs]
    jax.block_until_ready(concat_in)

    def run_chain(N):
        z = list(concat_zeros)
        t0 = time.perf_counter()
        for _ in range(N):
            z = list(sharded(*concat_in, *z))
        jax.block_until_ready(z)
        return time.perf_counter() - t0

    run_chain(2)  # warm-up / compile
    mins = {}
    for N in (n_small, n_big):
        mins[N] = min(run_chain(N) for _ in range(reps))
    slope = (mins[n_big] - mins[n_small]) / (n_big - n_small)
    return slope, mins


def kernel(**inputs):
    out, _ = run_on_cores(inputs, trace=False)
    return out
